# revision 15
# baseline (speedup 1.0000x reference)
"""Trainium2 Bass kernel for a 2-layer GraphSAGE(mean) encoder (8 NeuronCores).

Sharding: dst-node partition by (node_id % 8) for both layers.
  - Layer 0: core c owns dst0 nodes {d : d % 8 == c} (6250 nodes -> 49 tiles of
    128 local rows, processed in groups of 4 tiles = 512 PSUM columns).  Each
    core gathers x[src] rows for its incoming edges via dma_gather (int16
    indices, 7 base ranges of 32768 rows, contiguous span packing so pad rows
    only occur at span ends), applies log1p (f32 -> fp16) per gathered span,
    and segment-sums via one-hot matmuls on the PE
    (aggT[f, 512d] += H_chunk^T @ M_chunk, M[e, d] = (dstloc[e]==d)*inv_cnt),
    fp16 operands with f32 PSUM accumulation.
  - Layer 1: edges are assigned to cores by src1 % 8 so every message row is
    core-local (h1 scratch is fp16); each core computes weighted partial
    segment sums over ALL 10000 dst1 nodes (permuted layout grouped by
    dst1 % 8) and a single fp16 ReduceScatter(add) delivers each core its own
    1250 dst1 rows.
  - Weights are replicated (fp16); the final projection / relu / L2-normalize /
    heads run on the owning core; the host interleaves per-core outputs back.

kernel(**inputs) takes the FULL inputs (as produced by reference.setup_inputs)
and returns (z_loc, z_scale) as float32 numpy arrays of shape [10000, 32].
"""

import math

import numpy as np

import concourse.bass as bass
import concourse.bacc as bacc
import concourse.mybir as mybir
from concourse.bass_utils import run_bass_kernel_spmd
from concourse.masks import make_identity
from concourse.tile import TileContext

# ----------------------------------------------------------------------------
# Problem constants (hardcoded; the harness always uses these shapes).
# ----------------------------------------------------------------------------
N0, N1, N2 = 200000, 50000, 10000
E0, E1 = 800000, 160000
F_IN, H, L = 128, 256, 32
NC = 8
P = 128
RANGE = 32768  # int16-addressable row range for dma_gather

T0 = math.ceil(N1 // NC / P)  # 49 layer-0 dst tiles per core (6272 local rows)
R0 = T0 * P  # 6272 padded local rows per core
NR0 = math.ceil(N0 / RANGE)  # 7 source ranges for layer-0 gather
GT = 4  # layer-0 tiles per PSUM group (512 dst slots)
B1 = math.ceil(N2 // NC / P) * P  # 1280: padded per-core block of dst1 nodes
T1P = NC * B1 // P  # 80 partial tiles (10240 permuted rows)
T1 = B1 // P  # 10 final tiles per core
HB1 = B1 // 2  # 640: half-block rows per core (split ReduceScatter)
EPS_NORM = 1e-12
PAD_DST = 600.0  # dstloc sentinel (>512) -> one-hot column of zeros

f32 = mybir.dt.float32
f16 = mybir.dt.float16
i16 = mybir.dt.int16

# Max rows per dma_gather instruction (Q7 idx scratch limit: >1024 crashes).
GCHUNKS = 8


def _ranks_from_sorted(keys_sorted):
    """rank of each element within its equal-key run (keys_sorted ascending)."""
    n = keys_sorted.shape[0]
    if n == 0:
        return np.zeros(0, np.int64)
    new_run = np.empty(n, bool)
    new_run[0] = True
    new_run[1:] = keys_sorted[1:] != keys_sorted[:-1]
    starts = np.flatnonzero(new_run)
    run_ids = np.cumsum(new_run) - 1
    return np.arange(n) - starts[run_ids]


class _Grid0:
    """Group-PSUM layout: groups of `gt` tiles, contiguous span packing per
    (group, range).  All cores share the chunk layout (caps = max over cores);
    edges fill a span's slots contiguously, pads only at span ends."""

    def __init__(self, core, tile, rng, rel, loc, n_tiles, n_ranges, gt):
        self.n_tiles = n_tiles
        self.n_ranges = n_ranges
        self.gt = gt
        self.n_groups = math.ceil(n_tiles / gt)
        group = tile // gt
        gloc = (tile % gt) * P + loc  # dst slot within the gt*128-wide group

        counts = np.zeros((NC, self.n_groups, n_ranges), np.int64)
        np.add.at(counts, (core, group, rng), 1)
        self.cap = np.ceil(counts.max(axis=0) / P).astype(np.int64)  # [G, R]

        # chunk/span layout (shared by all cores)
        self.span_start = np.zeros((self.n_groups, n_ranges), np.int64)
        self.gbase = []
        self.gsize = []
        pos = 0
        for g in range(self.n_groups):
            self.gbase.append(pos)
            for r in range(n_ranges):
                self.span_start[g, r] = pos
                pos += self.cap[g, r]
            self.gsize.append(pos - self.gbase[g])
        self.total_chunks = pos
        self.max_gsize = max(self.gsize)

        # idx cols per (g, r)
        self.idx_cols = np.zeros((self.n_groups, n_ranges), np.int64)
        cpos = 0
        for g in range(self.n_groups):
            for r in range(n_ranges):
                self.idx_cols[g, r] = cpos
                cpos += self.cap[g, r] * P // 16
        self.total_idx_cols = max(cpos, 1)

        # ---------------- per-core arrays ----------------
        self.idx = np.zeros((NC, 128, self.total_idx_cols), np.int16)
        self.dstw = np.zeros((NC, 128, max(self.total_chunks, 1), 2), np.float32)
        self.dstw[..., 0] = PAD_DST

        order = np.lexsort((rng, group, core))
        key = (core.astype(np.int64) * self.n_groups + group) * n_ranges + rng
        ranks = _ranks_from_sorted(key[order])
        e_chunk = self.span_start[group[order], rng[order]] + ranks // P
        e_part = (ranks % P).astype(np.int64)

        dst_lin = np.full((max(self.total_chunks, 1), P), PAD_DST, np.float32)
        w_lin = np.zeros((max(self.total_chunks, 1), P), np.float32)
        idx_lin = np.zeros((max(self.total_chunks, 1) * P,), np.int16)
        co = core[order]
        for c in range(NC):
            m = co == c
            dst_lin[:] = PAD_DST
            w_lin[:] = 0.0
            idx_lin[:] = 0
            dst_lin[e_chunk[m], e_part[m]] = gloc[order][m]
            idx_lin[e_chunk[m] * P + e_part[m]] = rel[order][m]
            self.dstw[c, :, :, 0] = dst_lin.T
            self.dstw[c, :, :, 1] = w_lin.T
            for g in range(self.n_groups):
                for r in range(n_ranges):
                    nchunk = self.cap[g, r]
                    if nchunk == 0:
                        continue
                    c0 = self.span_start[g, r]
                    seg = idx_lin[c0 * P : (c0 + nchunk) * P]
                    col0 = self.idx_cols[g, r]
                    ncols = nchunk * P // 16
                    wrapped = seg.reshape(ncols, 16).T  # [16, ncols]
                    self.idx[c, :, col0 : col0 + ncols] = np.tile(wrapped, (8, 1))

    def signature(self):
        return ("g0", self.gt, self.n_tiles, tuple(self.cap.ravel().tolist()))


def _preprocess(x, src0, dst0, src1, dst1):
    src0 = np.asarray(src0).astype(np.int64)
    dst0 = np.asarray(dst0).astype(np.int64)
    src1 = np.asarray(src1).astype(np.int64)
    dst1 = np.asarray(dst1).astype(np.int64)

    deg0 = np.bincount(dst0, minlength=N1)
    inv0 = (1.0 / np.maximum(deg0, 1)).astype(np.float32)
    deg1 = np.bincount(dst1, minlength=N2)
    inv1 = (1.0 / np.maximum(deg1, 1)).astype(np.float32)

    # Layer 0: partition by dst % 8
    g0 = _Grid0(
        core=dst0 % NC,
        tile=(dst0 // NC) // P,
        rng=src0 // RANGE,
        rel=(src0 % RANGE).astype(np.int16),
        loc=((dst0 // NC) % P).astype(np.float32),
        n_tiles=T0,
        n_ranges=NR0,
        gt=GT,
    )

    # Layer 1: partition edges by src % 8; dst in half-major permuted layout
    # pd = half*5120 + (dst1%8)*640 + (dst1//8 - half*640) so the first 40
    # partial tiles cover every core's first half-block (split ReduceScatter).
    j1 = dst1 // NC
    hh = j1 // HB1
    pd = hh * (NC * HB1) + (dst1 % NC) * HB1 + (j1 - hh * HB1)
    g1 = _Grid0(
        core=src1 % NC,
        tile=pd // P,
        rng=np.zeros(E1, np.int64),
        rel=(src1 // NC).astype(np.int16),
        loc=(pd % P).astype(np.float32),
        n_tiles=T1P,
        n_ranges=1,
        gt=GT,
    )

    # Per-core self rows of x (the core's own dst0 partition), padded to R0.
    x = np.asarray(x, dtype=np.float32)
    xself = np.zeros((NC, R0, F_IN), np.float32)
    for c in range(NC):
        rows = x[c::NC][: N1 // NC]
        xself[c, : rows.shape[0]] = rows

    # Broadcast inverse-degree tables (weights applied post-aggregation).
    ngw = g0.n_groups * GT * P
    inv0b = np.ones((NC, ngw), np.float32)
    for c in range(NC):
        ell = np.arange(ngw)  # local dst index within the core
        d = ell * NC + c
        valid = ell < N1 // NC
        inv0b[c, valid] = inv0[d[valid]]
    pdv = np.arange(T1P * P)
    h = pdv // (NC * HB1)
    rem = pdv % (NC * HB1)
    c1 = rem // HB1
    j = h * HB1 + rem % HB1
    inv1b = np.ones(T1P * P, np.float32)
    valid = j < N2 // NC
    inv1b[valid] = inv1[(j * NC + c1)[valid]]
    return g0, g1, xself, inv0b, inv1b


# ----------------------------------------------------------------------------
# Program construction
# ----------------------------------------------------------------------------
def _build_program(g0, g1, has_b0, has_b1, has_bmu, has_bvar):
    nc = bacc.Bacc(num_devices=NC, name="gnn_sage")

    x_d = nc.dram_tensor("x", (N0, F_IN), f32, kind="ExternalInput")
    xself_d = nc.dram_tensor("xself", (R0, F_IN), f32, kind="ExternalInput")
    ws0_d = nc.dram_tensor("W_self0", (F_IN, H), f16, kind="ExternalInput")
    wn0_d = nc.dram_tensor("W_neigh0", (F_IN, H), f16, kind="ExternalInput")
    ws1_d = nc.dram_tensor("W_self1", (H, H), f16, kind="ExternalInput")
    wn1_d = nc.dram_tensor("W_neigh1", (H, H), f16, kind="ExternalInput")
    wmu_d = nc.dram_tensor("W_mu", (H, L), f16, kind="ExternalInput")
    wvar_d = nc.dram_tensor("W_var", (H, L), f16, kind="ExternalInput")
    iota_d = nc.dram_tensor("iota512", (P, GT * P), f16, kind="ExternalInput")
    inv0_d = nc.dram_tensor("inv0b", (P, g0.n_groups * GT * P), f32, kind="ExternalInput")
    inv1_d = nc.dram_tensor("inv1b", (P, T1P * P), f32, kind="ExternalInput")
    l0_idx_d = nc.dram_tensor("l0_idx", (128, g0.total_idx_cols), i16, kind="ExternalInput")
    l0_dstw_d = nc.dram_tensor("l0_dstw", (128, g0.total_chunks, 2), f32, kind="ExternalInput")
    l1_idx_d = nc.dram_tensor("l1_idx", (128, g1.total_idx_cols), i16, kind="ExternalInput")
    l1_dstw_d = nc.dram_tensor("l1_dstw", (128, g1.total_chunks, 2), f32, kind="ExternalInput")
    b_d = {}
    if has_b0:
        b_d["b0"] = nc.dram_tensor("b0", (H,), f16, kind="ExternalInput")
    if has_b1:
        b_d["b1"] = nc.dram_tensor("b1", (H,), f16, kind="ExternalInput")
    if has_bmu:
        b_d["b_mu"] = nc.dram_tensor("b_mu", (L,), f16, kind="ExternalInput")
    if has_bvar:
        b_d["b_var"] = nc.dram_tensor("b_var", (L,), f16, kind="ExternalInput")

    h1_d = nc.dram_tensor("h1_scratch", (R0, H), f16, kind="Internal")
    partials_a_d = nc.dram_tensor("s1_partials_a", (T1P * P // 2, H), f16, kind="Internal")
    partials_b_d = nc.dram_tensor("s1_partials_b", (T1P * P // 2, H), f16, kind="Internal")
    rs_a_d = nc.dram_tensor("s1_reduced_a", (HB1, H), f16, kind="Internal")
    rs_b_d = nc.dram_tensor("s1_reduced_b", (HB1, H), f16, kind="Internal")

    zloc_d = nc.dram_tensor("z_loc", (B1, L), f32, kind="ExternalOutput")
    zscale_d = nc.dram_tensor("z_scale", (B1, L), f32, kind="ExternalOutput")

    AT = mybir.ActivationFunctionType
    OP = mybir.AluOpType

    with TileContext(nc, num_cores=NC) as tc:
        with (
            tc.tile_pool(name="const", bufs=1) as cp,
            tc.tile_pool(name="span", bufs=6) as spanp,
            tc.tile_pool(name="stage16", bufs=2) as stage16p,
            tc.tile_pool(name="l1stage", bufs=2) as l1stagep,
            tc.tile_pool(name="meta", bufs=6) as metap,
            tc.tile_pool(name="onehot", bufs=6) as mp,
            tc.tile_pool(name="small", bufs=4) as sp,
            tc.tile_pool(name="selfp", bufs=4) as selfp,
            tc.tile_pool(name="ps_seg", bufs=2, space="PSUM") as ps_seg,
            tc.tile_pool(name="ps_tr", bufs=2, space="PSUM") as ps_tr,
            tc.tile_pool(name="ps_out", bufs=2, space="PSUM") as ps_out,
        ):
            # ---- constants ----
            iota_sb = cp.tile([P, GT * P], f16)
            nc.scalar.dma_start(out=iota_sb[:], in_=iota_d[:])
            inv0_sb = cp.tile([P, g0.n_groups * GT * P], f32)
            nc.scalar.dma_start(out=inv0_sb[:], in_=inv0_d[:])
            inv1_sb = cp.tile([P, T1P * P], f32)
            nc.scalar.dma_start(out=inv1_sb[:], in_=inv1_d[:])
            ident_sb = cp.tile([P, P], f16)
            make_identity(nc, ident_sb[:])
            ws0_sb = cp.tile([P, H], f16)
            nc.scalar.dma_start(out=ws0_sb[:], in_=ws0_d[:])
            wn0_sb = cp.tile([P, H], f16)
            nc.scalar.dma_start(out=wn0_sb[:], in_=wn0_d[:])
            ws1_sb = [cp.tile([P, H], f16, tag=f"ws1_{k}", name=f"ws1_{k}") for k in range(2)]
            wn1_sb = [cp.tile([P, H], f16, tag=f"wn1_{k}", name=f"wn1_{k}") for k in range(2)]
            wmu_sb = [cp.tile([P, L], f16, tag=f"wmu_{k}", name=f"wmu_{k}") for k in range(2)]
            wvar_sb = [cp.tile([P, L], f16, tag=f"wvar_{k}", name=f"wvar_{k}") for k in range(2)]
            for k in range(2):
                sl = slice(k * P, (k + 1) * P)
                nc.scalar.dma_start(out=ws1_sb[k][:], in_=ws1_d[sl, :])
                nc.scalar.dma_start(out=wn1_sb[k][:], in_=wn1_d[sl, :])
                nc.scalar.dma_start(out=wmu_sb[k][:], in_=wmu_d[sl, :])
                nc.scalar.dma_start(out=wvar_sb[k][:], in_=wvar_d[sl, :])
            zero_sb = cp.tile([P, H], f16)
            nc.vector.memset(zero_sb[:], 0.0)
            if b_d:
                ones_sb = cp.tile([1, P], f16)
                nc.vector.memset(ones_sb[:], 1.0)
                brow = {}
                for name, hd in b_d.items():
                    t = cp.tile([1, hd.shape[0]], f16, tag=f"brow_{name}", name=f"brow_{name}")
                    nc.sync.dma_start(out=t[:], in_=hd[:].rearrange("n -> 1 n"))
                    brow[name] = t

            x_ap = x_d[:]

            # ================= Layer 0 =================
            for g in range(g0.n_groups):
                sg = g0.gsize[g]
                gb = g0.gbase[g]
                stage16 = stage16p.tile([P, g0.max_gsize * P], f16, tag="st16")
                st16_3 = stage16[:].rearrange("p (s e) -> p s e", e=P)
                dstw_sb = metap.tile([128, g0.max_gsize, 2], f32, tag="dstw")
                nc.sync.dma_start(
                    out=dstw_sb[:, :sg, :], in_=l0_dstw_d[:, gb : gb + sg, :]
                )

                for r in range(NR0):
                    nchunk = g0.cap[g, r]
                    if nchunk == 0:
                        continue
                    col0 = g0.idx_cols[g, r]
                    row_lo = r * RANGE
                    row_hi = min((r + 1) * RANGE, N0)
                    for sub in range(0, nchunk, GCHUNKS):
                        k = min(GCHUNKS, nchunk - sub)
                        lc = g0.span_start[g, r] - gb + sub
                        span = spanp.tile([P, GCHUNKS, P], f32, tag="span")
                        idx_sb = metap.tile([128, GCHUNKS * P // 16], i16, tag="idx")
                        nc.sync.dma_start(
                            out=idx_sb[:, : k * P // 16],
                            in_=l0_idx_d[
                                :, col0 + sub * (P // 16) : col0 + (sub + k) * (P // 16)
                            ],
                        )
                        nreg = nc.gpsimd.to_reg(k * P)
                        nc.gpsimd.dma_gather(
                            out_ap=span[:, :k, :],
                            in_ap=x_ap[row_lo:row_hi, :],
                            idxs_ap=idx_sb[:, : k * P // 16],
                            num_idxs=k * P,
                            num_idxs_reg=nreg,
                            elem_size=F_IN,
                            queue_num=0,
                        )
                        nc.gpsimd.free_register(nreg)
                        # log1p f32 -> fp16 into the group slab
                        nc.scalar.activation(
                            st16_3[:, lc : lc + k, :].rearrange("p s e -> p (s e)"),
                            span[:, :k, :].rearrange("p s e -> p (s e)"),
                            AT.Ln,
                            bias=1.0,
                        )

                # one-hot matmuls: whole group accumulates into one [128, 512]
                ps_a = ps_seg.tile([P, GT * P], f32, tag="ps_a", name="ps_a")
                for j in range(sg):
                    m = mp.tile([P, GT * P], f16, tag="m")
                    nc.vector.tensor_scalar(
                        out=m[:],
                        in0=iota_sb[:],
                        scalar1=dstw_sb[:, j, 0:1],
                        scalar2=None,
                        op0=OP.is_equal,
                    )
                    nc.tensor.matmul(
                        out=ps_a[:],
                        lhsT=st16_3[:, j, :],
                        rhs=m[:],
                        start=(j == 0),
                        stop=(j == sg - 1),
                    )
                aggT_sb = sp.tile([P, GT * P], f16, tag="aggT")
                nc.vector.tensor_tensor(
                    out=aggT_sb[:],
                    in0=ps_a[:],
                    in1=inv0_sb[:, g * GT * P : (g + 1) * GT * P],
                    op=OP.mult,
                )

                for ti, t in enumerate(range(g * GT, min((g + 1) * GT, T0))):
                    # self rows -> log1p (fp16) -> transpose
                    self_sb = selfp.tile([P, F_IN], f32, tag="self0")
                    nc.sync.dma_start(out=self_sb[:], in_=xself_d[t * P : (t + 1) * P, :])
                    self16_sb = selfp.tile([P, F_IN], f16, tag="self0_16")
                    nc.scalar.activation(self16_sb[:], self_sb[:], AT.Ln, bias=1.0)
                    ps_t = ps_tr.tile([P, P], f16, tag="ps_t", name="ps_t")
                    nc.tensor.transpose(out=ps_t[:], in_=self16_sb[:], identity=ident_sb[:])
                    hdT_sb = sp.tile([P, P], f16, tag="hdT")
                    nc.vector.tensor_copy(out=hdT_sb[:], in_=ps_t[:])

                    ps_o = ps_out.tile([P, H], f32, tag="ps_o", name="ps_o")
                    nc.tensor.matmul(out=ps_o[:], lhsT=hdT_sb[:], rhs=ws0_sb[:], start=True, stop=False)
                    nc.tensor.matmul(
                        out=ps_o[:],
                        lhsT=aggT_sb[:, ti * P : (ti + 1) * P],
                        rhs=wn0_sb[:],
                        start=False,
                        stop=not has_b0,
                    )
                    if has_b0:
                        nc.tensor.matmul(
                            out=ps_o[:], lhsT=ones_sb[:], rhs=brow["b0"][:], start=False, stop=True
                        )
                    h1p = sp.tile([P, H], f32, tag="h1p")
                    nc.scalar.activation(h1p[:], ps_o[:], AT.Relu)
                    sq = sp.tile([P, H], f32, tag="sq")
                    ss = sp.tile([P, 1], f32, tag="ss")
                    nc.scalar.activation(sq[:], h1p[:], AT.Square, accum_out=ss[:])
                    nrm = sp.tile([P, 1], f32, tag="nrm")
                    nc.scalar.activation(nrm[:], ss[:], AT.Sqrt)
                    nrm2 = sp.tile([P, 1], f32, tag="nrm2")
                    nc.vector.tensor_scalar_max(nrm2[:], nrm[:], EPS_NORM)
                    rinv = sp.tile([P, 1], f32, tag="rinv")
                    nc.vector.reciprocal(rinv[:], nrm2[:])
                    h1n = sp.tile([P, H], f16, tag="h1n")
                    nc.vector.tensor_scalar(
                        out=h1n[:], in0=h1p[:], scalar1=rinv[:, 0:1], scalar2=None, op0=OP.mult
                    )
                    nc.sync.dma_start(out=h1_d[t * P : (t + 1) * P, :], in_=h1n[:])

            # ================= Layer 1 partial segment sums =================
            # aggT orientation: per group of 4 tiles, two [128f, 512pd] PSUMs
            # (feature halves) accumulate over the group's chunks; transposed
            # to pd-major fp16 partials.  Groups 0..9 cover every core's first
            # half-block -> ReduceScatter A fires while groups 10..19 compute.
            h1_ap = h1_d[:]
            for g in range(g1.n_groups):
                sg = g1.gsize[g]
                gb = g1.gbase[g]
                stage = l1stagep.tile([P, g1.max_gsize * H], f16, tag="l1st")
                stage3 = stage[:].rearrange("p (s e) -> p s e", e=H)
                dstw_sb = metap.tile([128, g1.max_gsize, 2], f32, tag="dstw1")
                nc.sync.dma_start(out=dstw_sb[:, :sg, :], in_=l1_dstw_d[:, gb : gb + sg, :])

                nchunk = g1.cap[g, 0]
                col0 = g1.idx_cols[g, 0]
                for sub in range(0, nchunk, GCHUNKS):
                    k = min(GCHUNKS, nchunk - sub)
                    idx_sb = metap.tile([128, GCHUNKS * P // 16], i16, tag="idx")
                    nc.sync.dma_start(
                        out=idx_sb[:, : k * P // 16],
                        in_=l1_idx_d[
                            :, col0 + sub * (P // 16) : col0 + (sub + k) * (P // 16)
                        ],
                    )
                    nreg = nc.gpsimd.to_reg(k * P)
                    nc.gpsimd.dma_gather(
                        out_ap=stage3[:, sub : sub + k, :],
                        in_ap=h1_ap,
                        idxs_ap=idx_sb[:, : k * P // 16],
                        num_idxs=k * P,
                        num_idxs_reg=nreg,
                        elem_size=H,
                        queue_num=0,
                    )
                    nc.gpsimd.free_register(nreg)

                ps_h = [
                    ps_seg.tile([P, GT * P], f32, tag="ps_a", name="ps_b0"),
                    ps_seg.tile([P, GT * P], f32, tag="ps_b1", name="ps_b1"),
                ]
                for j in range(sg):
                    m = mp.tile([P, GT * P], f16, tag="m")
                    nc.vector.tensor_scalar(
                        out=m[:],
                        in0=iota_sb[:],
                        scalar1=dstw_sb[:, j, 0:1],
                        scalar2=None,
                        op0=OP.is_equal,
                    )
                    for half in range(2):
                        nc.tensor.matmul(
                            out=ps_h[half][:],
                            lhsT=stage3[:, j, half * P : (half + 1) * P],
                            rhs=m[:],
                            start=(j == 0),
                            stop=(j == sg - 1),
                        )
                # inverse-degree scale (per pd column) + fp16 cast
                aggTh = []
                for half in range(2):
                    a = sp.tile([P, GT * P], f16, tag=f"aggTh{half}")
                    nc.vector.tensor_tensor(
                        out=a[:],
                        in0=ps_h[half][:],
                        in1=inv1_sb[:, g * GT * P : (g + 1) * GT * P],
                        op=OP.mult,
                    )
                    aggTh.append(a)
                # transpose to pd-major partials [512, 256] and store
                pdst = partials_a_d if g < g1.n_groups // 2 else partials_b_d
                prow = (g if g < g1.n_groups // 2 else g - g1.n_groups // 2) * GT * P
                for b in range(GT):
                    part_sb = sp.tile([P, H], f16, tag="part")
                    for half in range(2):
                        ps_t = ps_tr.tile([P, P], f16, tag="ps_t", name="ps_t")
                        nc.tensor.transpose(
                            out=ps_t[:],
                            in_=aggTh[half][:, b * P : (b + 1) * P],
                            identity=ident_sb[:],
                        )
                        nc.vector.tensor_copy(
                            out=part_sb[:, half * P : (half + 1) * P], in_=ps_t[:]
                        )
                    nc.sync.dma_start(
                        out=pdst[prow + b * P : prow + (b + 1) * P, :], in_=part_sb[:]
                    )

                if g == g1.n_groups // 2 - 1:
                    nc.gpsimd.collective_compute(
                        kind="ReduceScatter",
                        op=OP.add,
                        replica_groups=[list(range(NC))],
                        ins=[partials_a_d[:]],
                        outs=[rs_a_d[:]],
                    )
            nc.gpsimd.collective_compute(
                kind="ReduceScatter",
                op=OP.add,
                replica_groups=[list(range(NC))],
                ins=[partials_b_d[:]],
                outs=[rs_b_d[:]],
            )

            # ================= Layer 1 final + heads =================
            for t in range(T1):
                rows = slice(t * P, (t + 1) * P)
                rs_sb = sp.tile([P, H], f16, tag="rs")
                if t < T1 // 2:
                    nc.sync.dma_start(out=rs_sb[:], in_=rs_a_d[rows, :])
                else:
                    hrows = slice((t - T1 // 2) * P, (t - T1 // 2 + 1) * P)
                    nc.sync.dma_start(out=rs_sb[:], in_=rs_b_d[hrows, :])
                hd_sb = selfp.tile([P, H], f16, tag="self1")
                nc.sync.dma_start(out=hd_sb[:], in_=h1_d[rows, :])

                aggT1 = []
                hdT1 = []
                for half in range(2):
                    hs = slice(half * P, (half + 1) * P)
                    ps_t = ps_tr.tile([P, P], f16, tag="ps_t", name="ps_t")
                    nc.tensor.transpose(out=ps_t[:], in_=rs_sb[:, hs], identity=ident_sb[:])
                    a = sp.tile([P, P], f16, tag=f"aggT1_{half}")
                    nc.vector.tensor_copy(out=a[:], in_=ps_t[:])
                    aggT1.append(a)
                    ps_t2 = ps_tr.tile([P, P], f16, tag="ps_t", name="ps_t2")
                    nc.tensor.transpose(out=ps_t2[:], in_=hd_sb[:, hs], identity=ident_sb[:])
                    hh = sp.tile([P, P], f16, tag=f"hdT1_{half}")
                    nc.vector.tensor_copy(out=hh[:], in_=ps_t2[:])
                    hdT1.append(hh)

                ps_o = ps_out.tile([P, H], f32, tag="ps_o", name="ps_o")
                nc.tensor.matmul(out=ps_o[:], lhsT=hdT1[0][:], rhs=ws1_sb[0][:], start=True, stop=False)
                nc.tensor.matmul(out=ps_o[:], lhsT=hdT1[1][:], rhs=ws1_sb[1][:], start=False, stop=False)
                nc.tensor.matmul(out=ps_o[:], lhsT=aggT1[0][:], rhs=wn1_sb[0][:], start=False, stop=False)
                nc.tensor.matmul(
                    out=ps_o[:], lhsT=aggT1[1][:], rhs=wn1_sb[1][:], start=False, stop=not has_b1
                )
                if has_b1:
                    nc.tensor.matmul(
                        out=ps_o[:], lhsT=ones_sb[:], rhs=brow["b1"][:], start=False, stop=True
                    )
                h2p = sp.tile([P, H], f32, tag="h2p")
                nc.scalar.activation(h2p[:], ps_o[:], AT.Relu)
                sq = sp.tile([P, H], f32, tag="sq")
                ss = sp.tile([P, 1], f32, tag="ss")
                nc.scalar.activation(sq[:], h2p[:], AT.Square, accum_out=ss[:])
                nrm = sp.tile([P, 1], f32, tag="nrm")
                nc.scalar.activation(nrm[:], ss[:], AT.Sqrt)
                nrm2 = sp.tile([P, 1], f32, tag="nrm2")
                nc.vector.tensor_scalar_max(nrm2[:], nrm[:], EPS_NORM)
                rinv = sp.tile([P, 1], f32, tag="rinv")
                nc.vector.reciprocal(rinv[:], nrm2[:])
                h2n = sp.tile([P, H], f16, tag="h2n")
                nc.vector.tensor_scalar(
                    out=h2n[:], in0=h2p[:], scalar1=rinv[:, 0:1], scalar2=None, op0=OP.mult
                )

                h2T = []
                for half in range(2):
                    hs = slice(half * P, (half + 1) * P)
                    ps_t = ps_tr.tile([P, P], f16, tag="ps_t", name="ps_t")
                    nc.tensor.transpose(out=ps_t[:], in_=h2n[:, hs], identity=ident_sb[:])
                    hh = sp.tile([P, P], f16, tag=f"h2T_{half}")
                    nc.vector.tensor_copy(out=hh[:], in_=ps_t[:])
                    h2T.append(hh)

                ps_zl = ps_out.tile([P, L], f32, tag="ps_o", name="ps_zl")
                nc.tensor.matmul(out=ps_zl[:], lhsT=h2T[0][:], rhs=wmu_sb[0][:], start=True, stop=False)
                nc.tensor.matmul(
                    out=ps_zl[:], lhsT=h2T[1][:], rhs=wmu_sb[1][:], start=False, stop=not has_bmu
                )
                if has_bmu:
                    nc.tensor.matmul(
                        out=ps_zl[:], lhsT=ones_sb[:], rhs=brow["b_mu"][:], start=False, stop=True
                    )
                zl_sb = sp.tile([P, L], f32, tag="zl")
                nc.vector.tensor_copy(out=zl_sb[:], in_=ps_zl[:])
                nc.sync.dma_start(out=zloc_d[rows, :], in_=zl_sb[:])

                ps_zs = ps_out.tile([P, L], f32, tag="ps_o", name="ps_zs")
                nc.tensor.matmul(out=ps_zs[:], lhsT=h2T[0][:], rhs=wvar_sb[0][:], start=True, stop=False)
                nc.tensor.matmul(
                    out=ps_zs[:], lhsT=h2T[1][:], rhs=wvar_sb[1][:], start=False, stop=not has_bvar
                )
                if has_bvar:
                    nc.tensor.matmul(
                        out=ps_zs[:], lhsT=ones_sb[:], rhs=brow["b_var"][:], start=False, stop=True
                    )
                zs_sb = sp.tile([P, L], f32, tag="zs")
                nc.scalar.activation(zs_sb[:], ps_zs[:], AT.Exp)
                nc.vector.tensor_scalar_add(zs_sb[:], zs_sb[:], 1e-6)
                nc.sync.dma_start(out=zscale_d[rows, :], in_=zs_sb[:])

    nc.compile()
    return nc


# ----------------------------------------------------------------------------
# Entry point
# ----------------------------------------------------------------------------
_CACHE = {}


def prepare(inputs):
    """Host preprocessing + program build.  Returns (nc, in_maps, postprocess)."""
    x = np.asarray(inputs["x"], np.float32)
    g0, g1, xself, inv0b, inv1b = _preprocess(x, inputs["src0"], inputs["dst0"], inputs["src1"], inputs["dst1"])

    b0 = np.asarray(inputs["b0"], np.float32)
    b1 = np.asarray(inputs["b1"], np.float32)
    bmu = np.asarray(inputs["b_mu"], np.float32)
    bvar = np.asarray(inputs["b_var"], np.float32)
    has_b0, has_b1 = bool(np.any(b0)), bool(np.any(b1))
    has_bmu, has_bvar = bool(np.any(bmu)), bool(np.any(bvar))

    key = (g0.signature(), g1.signature(), has_b0, has_b1, has_bmu, has_bvar)
    if key not in _CACHE:
        _CACHE[key] = _build_program(g0, g1, has_b0, has_b1, has_bmu, has_bvar)
    nc = _CACHE[key]

    iota = np.broadcast_to(np.arange(GT * P, dtype=np.float16), (P, GT * P)).copy()
    common = {
        "x": x,
        "W_self0": np.asarray(inputs["W_self0"], np.float32).astype(np.float16),
        "W_neigh0": np.asarray(inputs["W_neigh0"], np.float32).astype(np.float16),
        "W_self1": np.asarray(inputs["W_self1"], np.float32).astype(np.float16),
        "W_neigh1": np.asarray(inputs["W_neigh1"], np.float32).astype(np.float16),
        "W_mu": np.asarray(inputs["W_mu"], np.float32).astype(np.float16),
        "W_var": np.asarray(inputs["W_var"], np.float32).astype(np.float16),
        "iota512": iota,
        "inv1b": np.broadcast_to(inv1b[None, :], (P, T1P * P)).copy(),
    }
    if has_b0:
        common["b0"] = b0.astype(np.float16)
    if has_b1:
        common["b1"] = b1.astype(np.float16)
    if has_bmu:
        common["b_mu"] = bmu.astype(np.float16)
    if has_bvar:
        common["b_var"] = bvar.astype(np.float16)

    in_maps = []
    for c in range(NC):
        m = dict(common)
        m["xself"] = xself[c]
        m["inv0b"] = np.broadcast_to(inv0b[c][None, :], (P, inv0b.shape[1])).copy()
        m["l0_idx"] = g0.idx[c]
        m["l0_dstw"] = g0.dstw[c]
        m["l1_idx"] = g1.idx[c]
        m["l1_dstw"] = g1.dstw[c]
        in_maps.append(m)

    def postprocess(results):
        z_loc = np.empty((N2, L), np.float32)
        z_scale = np.empty((N2, L), np.float32)
        nvalid = N2 // NC
        for c in range(NC):
            z_loc[c::NC] = results[c]["z_loc"][:nvalid]
            z_scale[c::NC] = results[c]["z_scale"][:nvalid]
        return z_loc, z_scale

    return nc, in_maps, postprocess


def kernel(**inputs):
    assert int(inputs.get("n_dst0", N1)) == N1 and int(inputs.get("n_dst1", N2)) == N2
    nc, in_maps, postprocess = prepare(inputs)
    res = run_bass_kernel_spmd(nc, in_maps, core_ids=list(range(NC)))
    return postprocess(res.results)
